# revision 1
# baseline (speedup 1.0000x reference)
"""Bass/Trainium2 kernel for nn_LocalAggregator (GNN message passing).

Math per batch b (hidden [64,128], adj [64,64] in {0..4}, a [4,128]):
    e_k[i,j] = leakyrelu_{0.2}( sum_d hidden[i,d]*hidden[j,d]*a[k,d] )
    alpha    = softmax_j( where(adj==k+1, e_k, -9e15) )
    out      = alpha @ hidden

Device strategy (8 cores, pure batch data-parallel, 64 batches/core,
processed in "quads" of 4 batches):
  - e_k is SYMMETRIC in (i,j).  We exploit this: the PSUM tile holding
    e_k[i,j] can be reinterpreted as e_k[j,i], so masking it with the
    host-TRANSPOSED adjacency produces the transposed attention weights
    w^T[j,i] directly -- no on-chip transposes anywhere.
  - leaky-relu runs on the ACT engine as Prelu(alpha=0.2) while it
    evacuates PSUM; Exp follows as a second ACT pass.
  - Selection is a multiplicative one-hot: w = (adjT==k+1) * exp(...).
    Masked entries become exactly 0, matching exp(-9e15 - max) == 0.
  - A ones-column appended to hidden makes the final matmul emit the
    softmax denominator s_i alongside alpha@h; normalize by 1/s_i after.
  - Host pre-packs bf16 layouts; matmuls in bf16 (fp32 PSUM accumulate).
"""

import numpy as np
import ml_dtypes

from contextlib import ExitStack

import concourse.bass as bass
import concourse.tile as tile
from concourse import bacc, mybir
from concourse._compat import with_exitstack
from concourse.bass_utils import run_bass_kernel_spmd

BF16 = mybir.dt.bfloat16
F32 = mybir.dt.float32
ALU = mybir.AluOpType
ACTF = mybir.ActivationFunctionType

B, N, D, K = 512, 64, 128, 4
NCORES = 8
BPC = B // NCORES          # 64 batches per core
QUADS = BPC // 4           # 16 quads of 4 batches per core
HHW = 132                  # hidden cols + ones col + pad (128 data, 1 ones, 3 zero)


@with_exitstack
def _kernel_body(ctx, tc, hT_d, hh_d, adjT_d, aT_d, out_d):
    nc = tc.nc

    const_pool = ctx.enter_context(tc.tile_pool(name="const", bufs=1))
    in_pool = ctx.enter_context(tc.tile_pool(name="inp", bufs=3))
    work_pool = ctx.enter_context(tc.tile_pool(name="work", bufs=3))
    psum_pool = ctx.enter_context(tc.tile_pool(name="psum", bufs=2, space="PSUM"))
    opsum_pool = ctx.enter_context(tc.tile_pool(name="opsum", bufs=2, space="PSUM"))
    out_pool = ctx.enter_context(tc.tile_pool(name="outp", bufs=3))

    # --- one-time constants ---
    a_sb = const_pool.tile([128, 4], F32)          # a^T : [d, k]
    nc.sync.dma_start(out=a_sb[:], in_=aT_d[:, :])
    # kpat[:, p*256 + k*64 + c] = k+1  (compare target for the one-hot)
    kpat = const_pool.tile([128, 512], BF16)
    for p in range(2):
        for k in range(K):
            nc.gpsimd.memset(kpat[:, p * 256 + k * 64 : p * 256 + (k + 1) * 64],
                             float(k + 1))

    for q in range(QUADS):
        # ---- loads ----
        # hT [128=d, 256=(l,i)] for the 4 batches l=0..3 of this quad
        hT = in_pool.tile([128, 256], BF16, tag="hT")
        nc.sync.dma_start(out=hT[:], in_=hT_d[q])
        # adjT [128=(u,r), 128=(p,c)] = adj[4q+2p+u][c, r]
        adjT = in_pool.tile([128, 128], BF16, tag="adjT")
        nc.sync.dma_start(out=adjT[:], in_=adjT_d[q])
        # hh[p] [128=(u,j), 132] original-layout hidden rows + ones col
        hh = []
        for p in range(2):
            t = in_pool.tile([128, HHW], BF16, tag=f"hh{p}")
            nc.sync.dma_start(
                out=t[:],
                in_=hh_d[4 * q + 2 * p : 4 * q + 2 * p + 2].flatten_outer_dims(),
            )
            hh.append(t)

        # ---- w_all[d, (l,k,j)] = hT[d, (l,j)] * a[k,d] ----
        # 4 per-k tensor_scalar ops on the (otherwise idle) Pool engine.
        w_all = work_pool.tile([128, 1024], BF16, tag="w_all")
        hTv = hT[:].rearrange("p (l j) -> p l j", l=4)
        w_allv = w_all[:].rearrange("p (l k j) -> p l k j", l=4, k=4)
        for k in range(K):
            nc.gpsimd.tensor_scalar(
                w_allv[:, :, k, :], hTv, a_sb[:, k : k + 1], None, ALU.mult)

        # ---- e4[(u,i), (p,k,j)] = e_k^{l=2p+u}[i,j] : 4 matmuls, K=d=128 ----
        e4 = psum_pool.tile([128, 512], F32, tag="e4")
        for l in range(4):
            p, u = l // 2, l % 2
            nc.tensor.matmul(
                e4[u * 64 : (u + 1) * 64, p * 256 : (p + 1) * 256],
                lhsT=hT[:, l * 64 : (l + 1) * 64],
                rhs=w_all[:, l * 256 : (l + 1) * 256],
                start=True, stop=True,
                tile_position=(0, u * 64),
            )

        # ---- xm = exp(leakyrelu(e)) : Prelu evacuates PSUM, then Exp ----
        lr4 = work_pool.tile([128, 512], F32, tag="lr4")
        nc.scalar.activation(lr4[:], e4[:], ACTF.Prelu, alpha=0.2)
        xm = work_pool.tile([128, 512], BF16, tag="xm")
        nc.scalar.activation(xm[:], lr4[:], ACTF.Exp)

        # ---- one-hot select via transposed adj (symmetry trick) ----
        ind = work_pool.tile([128, 512], BF16, tag="ind")
        adjv = (adjT[:].rearrange("p (t c) -> p t c", t=2)
                .unsqueeze(2).broadcast_to([128, 2, 4, 64]))
        kv = kpat[:].rearrange("p (t k c) -> p t k c", t=2, k=4)
        nc.vector.tensor_tensor(
            ind[:].rearrange("p (t k c) -> p t k c", t=2, k=4),
            adjv, kv, ALU.is_equal)
        w4 = work_pool.tile([128, 512], BF16, tag="w4")
        nc.vector.tensor_mul(w4[:], xm[:], ind[:])

        # ---- sum over k: w_sumT[(u,j), (p,i)] ----
        w4v = w4[:].rearrange("p (t k c) -> p t k c", t=2, k=4)
        t2 = work_pool.tile([128, 256], BF16, tag="t2")
        t2v = t2[:].rearrange("p (t k c) -> p t k c", t=2, k=2)
        nc.vector.tensor_tensor(t2v, w4v[:, :, 0:2, :], w4v[:, :, 2:4, :], ALU.add)
        wsum = work_pool.tile([128, 128], BF16, tag="wsum")
        wsv = wsum[:].rearrange("p (t c) -> p t c", t=2)
        nc.vector.tensor_tensor(wsv, t2v[:, :, 0, :], t2v[:, :, 1, :], ALU.add)

        # ---- out_p[(u,i), 0:128] = sum_j w^T[j,i] h[j,d]; col 128 = denom ----
        ops = []
        for p in range(2):
            t = opsum_pool.tile([128, HHW], F32, tag=f"ops{p}")
            ops.append(t)
        for l in range(4):
            p, u = l // 2, l % 2
            nc.tensor.matmul(
                ops[p][u * 64 : (u + 1) * 64, :],
                lhsT=wsum[u * 64 : (u + 1) * 64, p * 64 : (p + 1) * 64],
                rhs=hh[p][u * 64 : (u + 1) * 64, :],
                start=True, stop=True,
                tile_position=(u * 64, u * 64),
            )

        # ---- normalize rows by 1/denominator and store ----
        # (one scale on DVE, one on ACT to balance engine load)
        for p in range(2):
            r = work_pool.tile([128, 1], F32, tag=f"r{p}")
            nc.vector.reciprocal(r[:], ops[p][:, 128:129])
            osb = out_pool.tile([128, 128], F32, tag=f"osb{p}")
            if p == 0:
                nc.vector.tensor_scalar(osb[:], ops[p][:, 0:128], r[:], None, ALU.mult)
            else:
                nc.scalar.activation(osb[:], ops[p][:, 0:128], ACTF.Copy,
                                     scale=r[:])
            nc.sync.dma_start(
                out=out_d[4 * q + 2 * p : 4 * q + 2 * p + 2].flatten_outer_dims(),
                in_=osb[:],
            )


def build_nc():
    nc = bacc.Bacc("TRN2", target_bir_lowering=False, debug=False)
    hT_d = nc.dram_tensor("ht", [QUADS, 128, 256], BF16, kind="ExternalInput").ap()
    hh_d = nc.dram_tensor("hh", [BPC, 64, HHW], BF16, kind="ExternalInput").ap()
    adjT_d = nc.dram_tensor("adjt", [QUADS, 128, 128], BF16, kind="ExternalInput").ap()
    aT_d = nc.dram_tensor("at", [128, 4], F32, kind="ExternalInput").ap()
    out_d = nc.dram_tensor("out", [BPC, 64, 128], F32, kind="ExternalOutput").ap()
    with tile.TileContext(nc) as tc:
        _kernel_body(tc, hT_d, hh_d, adjT_d, aT_d, out_d)
    nc.compile()
    return nc


def prep_inputs(hidden, adj, a):
    """Host-side packing: bf16 casts, transposed/interleaved layouts, shards."""
    bf = ml_dtypes.bfloat16
    hidden = np.asarray(hidden, dtype=np.float32)
    adj = np.asarray(adj)
    a = np.asarray(a, dtype=np.float32)

    hb = hidden.astype(bf)                                   # [B, 64, 128]
    hh = np.zeros((B, N, HHW), dtype=bf)
    hh[:, :, 0:D] = hb
    hh[:, :, D] = bf(1.0)

    # hT_q[q, d, l*64+i] = hidden[4q+l, i, d]
    hT = (hb.transpose(0, 2, 1)                              # [B, d, i]
          .reshape(B // 4, 4, D, N)                          # [q, l, d, i]
          .transpose(0, 2, 1, 3)                             # [q, d, l, i]
          .reshape(B // 4, D, 4 * N))
    hT = np.ascontiguousarray(hT)

    # adjT_q[q, u*64+r, p*64+c] = adj[4q+2p+u][c, r]
    adjT = adj.transpose(0, 2, 1).astype(bf)                 # [b, r, c]
    adjTq = (adjT.reshape(B // 4, 2, 2, N, N)                # [q, p, u, r, c]
             .transpose(0, 2, 3, 1, 4)                       # [q, u, r, p, c]
             .reshape(B // 4, 2 * N, 2 * N))
    adjTq = np.ascontiguousarray(adjTq)

    aT = np.ascontiguousarray(a.T).astype(np.float32)        # [128, 4]

    in_maps = []
    for c in range(NCORES):
        bsl = slice(c * BPC, (c + 1) * BPC)
        qsl = slice(c * QUADS, (c + 1) * QUADS)
        in_maps.append({
            "ht": np.ascontiguousarray(hT[qsl]),
            "hh": np.ascontiguousarray(hh[bsl]),
            "adjt": np.ascontiguousarray(adjTq[qsl]),
            "at": aT,
        })
    return in_maps


_NC_CACHE = {}


def run_device(hidden, adj, a, **spmd_kwargs):
    if "nc" not in _NC_CACHE:
        _NC_CACHE["nc"] = build_nc()
    nc = _NC_CACHE["nc"]
    in_maps = prep_inputs(hidden, adj, a)
    res = run_bass_kernel_spmd(nc, in_maps, list(range(NCORES)), **spmd_kwargs)
    out = np.concatenate([res.results[c]["out"] for c in range(NCORES)], axis=0)
    return out.reshape(B, N, D).astype(np.float32), res


def kernel(hidden, adj, a):
    out, _ = run_device(hidden, adj, a)
    return out



# revision 3
# speedup vs baseline: 4.8051x; 4.8051x over previous
"""Bass/Trainium2 kernel for nn_LocalAggregator (GNN message passing).

Math per batch b (hidden [64,128], adj [64,64] in {0..4}, a [4,128]):
    e_k[i,j] = leakyrelu_{0.2}( sum_d hidden[i,d]*hidden[j,d]*a[k,d] )
    alpha    = softmax_j( where(adj==k+1, e_k, -9e15) )
    out      = alpha @ hidden

Device strategy (8 cores, pure batch data-parallel, 64 batches/core).
Batches are fused in PAIRS (2 batches = 128 nodes -> full-width matmuls;
cross-batch terms are computed but killed by the adjacency mask), and
processed in OCTs (4 pairs = 8 batches) so element-wise ops run on
[128, 2048] tiles that amortize per-op overheads.

Per oct q (tiles: hT [d, (pair,i)], hh [j2b, (pair, d+ones)],
adjT [j2b, (pair, i)] block-diagonal-transposed adjacency):
  - w_all[d,(pair,k,i)] = hT * a_k           (4x tensor_scalar, DVE 4x mode)
  - e2[j2b,(k,i)] = hT_pair^T @ w_all_pair   (1 matmul per pair, PSUM f32)
    e_k is symmetric, so this tile read as [j,(k,i)] is e_k[i,j].
  - lr = Prelu(e2) evacuates PSUM on ACT
  - A[j,(pair,k,i)] = (adjT != k+1) * -40    (4x dual-op tensor_scalar)
  - esel = max_k (lr + A): selected value where adj==k+1, else <= -35
    (leakyrelu commutes with one-hot selection; exp(-35) underflows to
    exactly 0 in fp16, which also kills the cross-batch block entries)
  - w = Exp(esel) on ACT ([128,512] only, 4x smaller than lr)
  - out_pair[i,d] = sum_j w[j,i]*hh[j,d]; ones-col gives denominator s_i
  - unnormalized out + s shipped fp16; host divides and casts.
"""

import numpy as np
import ml_dtypes

from contextlib import ExitStack

import concourse.bass as bass
import concourse.tile as tile
from concourse import bacc, mybir
from concourse._compat import with_exitstack
from concourse.bass_utils import run_bass_kernel_spmd

F16 = mybir.dt.float16
F32 = mybir.dt.float32
ALU = mybir.AluOpType
ACTF = mybir.ActivationFunctionType

B, N, D, K = 512, 64, 128, 4
NCORES = 8
BPC = B // NCORES          # 64 batches per core
NOCT = BPC // 8            # 8 octs of 8 batches (4 pairs) per core
HHW = 132                  # hidden cols + ones col + pad (128 data, 1 ones, 3 zero)
CIN = 512 + 4 * HHW + 512  # blob cols: hT(512) | hh(528) | adjT(512)
MASKV = -40.0              # additive mask; exp(-35) underflows fp16 -> 0


@with_exitstack
def _kernel_body(ctx, tc, blob_d, aT_d, out_d):
    nc = tc.nc

    const_pool = ctx.enter_context(tc.tile_pool(name="const", bufs=1))
    in_pool = ctx.enter_context(tc.tile_pool(name="inp", bufs=3))
    work_pool = ctx.enter_context(tc.tile_pool(name="work", bufs=2))
    psum_pool = ctx.enter_context(tc.tile_pool(name="psum", bufs=2, space="PSUM"))
    opsum_pool = ctx.enter_context(tc.tile_pool(name="opsum", bufs=2, space="PSUM"))
    out_pool = ctx.enter_context(tc.tile_pool(name="outp", bufs=3))

    a_sb = const_pool.tile([128, 4], F32)          # a^T : [d, k]
    nc.sync.dma_start(out=a_sb[:], in_=aT_d[:, :])

    for q in range(NOCT):
        blob = in_pool.tile([128, CIN], F16, tag="blob")
        nc.sync.dma_start(out=blob[:], in_=blob_d[q])
        hT = blob[:, 0:512]                       # [d, (pair, i)]
        hh = blob[:, 512 : 512 + 4 * HHW]         # [j2b, (pair, d+ones)]
        adjT = blob[:, 512 + 4 * HHW : CIN]       # [j2b, (pair, i)]

        # ---- w_all[d, (pair, k, i)] = hT * a_k ----
        w_all = work_pool.tile([128, 2048], F16, tag="w_all")
        wv = w_all[:].rearrange("p (a k i) -> p a k i", a=4, k=4)
        hTv = hT.rearrange("p (a i) -> p a i", a=4)
        for k in range(K):
            nc.vector.tensor_scalar(
                wv[:, :, k, :], hTv, a_sb[:, k : k + 1], None, ALU.mult)

        # ---- A[j, (pair, k, i)] = (adjT != k+1) * MASKV ----
        amask = work_pool.tile([128, 2048], F16, tag="amask")
        av = amask[:].rearrange("p (a k i) -> p a k i", a=4, k=4)
        adv = adjT.rearrange("p (a i) -> p a i", a=4)
        for k in range(K):
            nc.vector.tensor_scalar(
                av[:, :, k, :], adv, float(k + 1), MASKV,
                ALU.not_equal, ALU.mult)

        # ---- e2 per pair (PSUM f32), Prelu-evacuated to lr (fp16) ----
        lr = work_pool.tile([128, 2048], F16, tag="lr")
        for duo in range(2):
            e2 = psum_pool.tile([128, 1024], F32, tag="e2")
            for pp in range(2):
                p = 2 * duo + pp
                nc.tensor.matmul(
                    e2[:, pp * 512 : (pp + 1) * 512],
                    lhsT=hT[:, p * 128 : (p + 1) * 128],
                    rhs=w_all[:, p * 512 : (p + 1) * 512],
                    start=True, stop=True,
                )
            nc.scalar.activation(
                lr[:, duo * 1024 : (duo + 1) * 1024], e2[:],
                ACTF.Prelu, alpha=0.2)

        # ---- esel[j, (pair, i)] = max_k (lr + A) ----
        z = work_pool.tile([128, 2048], F16, tag="z")
        nc.vector.tensor_tensor(z[:], lr[:], amask[:], ALU.add)
        zv = z[:].rearrange("p (a k i) -> p a k i", a=4, k=4)
        t2 = work_pool.tile([128, 1024], F16, tag="t2")
        t2v = t2[:].rearrange("p (a k i) -> p a k i", a=4, k=2)
        nc.vector.tensor_tensor(t2v, zv[:, :, 0:2, :], zv[:, :, 2:4, :], ALU.max)
        esel = work_pool.tile([128, 512], F16, tag="esel")
        eselv = esel[:].rearrange("p (a i) -> p a i", a=4)
        nc.vector.tensor_tensor(eselv, t2v[:, :, 0, :], t2v[:, :, 1, :], ALU.max)

        # ---- w[j, (pair, i)] = exp(esel) : masked entries -> exactly 0 ----
        w = work_pool.tile([128, 512], F16, tag="w")
        nc.scalar.activation(w[:], esel[:], ACTF.Exp)

        # ---- out_pair[i, 0:128] = sum_j w[j,i] h[j,d]; col 128 = denom ----
        osum = opsum_pool.tile([128, 1024], F32, tag="osum")
        for p in range(4):
            nc.tensor.matmul(
                osum[:, p * 256 : p * 256 + HHW],
                lhsT=w[:, p * 128 : (p + 1) * 128],
                rhs=hh[:, p * HHW : (p + 1) * HHW],
                start=True, stop=True,
            )

        # ---- evacuate (unnormalized) to fp16 and store ----
        osb = out_pool.tile([128, 4 * HHW], F16, tag="osb")
        osbv = osb[:].rearrange("p (a c) -> p a c", a=4)
        osumv = osum[:].rearrange("p (a c) -> p a c", a=4)[:, :, 0:HHW]
        nc.scalar.activation(osbv, osumv, ACTF.Copy)
        nc.sync.dma_start(out=out_d[q], in_=osb[:])


def build_nc():
    nc = bacc.Bacc("TRN2", target_bir_lowering=False, debug=False)
    blob_d = nc.dram_tensor("blob", [NOCT, 128, CIN], F16, kind="ExternalInput").ap()
    aT_d = nc.dram_tensor("at", [128, 4], F32, kind="ExternalInput").ap()
    out_d = nc.dram_tensor("out", [NOCT, 128, 4 * HHW], F16,
                           kind="ExternalOutput").ap()
    with tile.TileContext(nc) as tc:
        _kernel_body(tc, blob_d, aT_d, out_d)
    nc.compile()
    return nc


def prep_inputs(hidden, adj, a):
    """Host-side packing: fp16 casts, pair-fused block layouts, shards."""
    hidden = np.asarray(hidden, dtype=np.float32)
    adj = np.asarray(adj)
    a = np.asarray(a, dtype=np.float32)

    h16 = hidden.astype(np.float16)                          # [B, 64, 128]

    # hT[b-pairs]: [d, (pair, v)] with v = (u*64 + i), batch = 2*pair_g + u
    hT = (h16.transpose(0, 2, 1)                             # [B, d, i]
          .reshape(B // 2, 2, D, N)                          # [pg, u, d, i]
          .transpose(0, 2, 1, 3)                             # [pg, d, u, i]
          .reshape(B // 2, D, 2 * N))                        # [pg, d, v]

    # hh[pg, v, c]: row v = h[2pg + v//64, v%64, :] plus ones col
    hh = np.zeros((B // 2, 2 * N, HHW), dtype=np.float16)
    hh[:, :, 0:D] = h16.reshape(B // 2, 2 * N, D)
    hh[:, :, D] = np.float16(1.0)

    # adjT block tile [pg, x, y] = adj[2pg + x//64, y%64, x%64] if same half
    adjT = np.zeros((B // 2, 2 * N, 2 * N), dtype=np.float16)
    at = adj.transpose(0, 2, 1).astype(np.float16)           # at[b, j, i]
    adjT[:, 0:N, 0:N] = at[0::2]
    adjT[:, N:2 * N, N:2 * N] = at[1::2]

    aT = np.ascontiguousarray(a.T).astype(np.float32)        # [128, 4]

    # blob[oct, 128, CIN] per core: hT(4 pairs) | hh(4 pairs) | adjT(4 pairs)
    PPC = BPC // 2                                           # 32 pairs per core
    in_maps = []
    for c in range(NCORES):
        psl = slice(c * PPC, (c + 1) * PPC)
        hT_c = hT[psl].reshape(NOCT, 4, D, 2 * N)            # [q, pair, d, v]
        hh_c = hh[psl].reshape(NOCT, 4, 2 * N, HHW)
        adjT_c = adjT[psl].reshape(NOCT, 4, 2 * N, 2 * N)
        blob = np.empty((NOCT, 128, CIN), dtype=np.float16)
        blob[:, :, 0:512] = hT_c.transpose(0, 2, 1, 3).reshape(NOCT, 128, 512)
        blob[:, :, 512:512 + 4 * HHW] = (
            hh_c.transpose(0, 2, 1, 3).reshape(NOCT, 128, 4 * HHW))
        blob[:, :, 512 + 4 * HHW:CIN] = (
            adjT_c.transpose(0, 2, 1, 3).reshape(NOCT, 128, 512))
        in_maps.append({"blob": np.ascontiguousarray(blob), "at": aT})
    return in_maps


_NC_CACHE = {}


def run_device(hidden, adj, a, **spmd_kwargs):
    if "nc" not in _NC_CACHE:
        _NC_CACHE["nc"] = build_nc()
    nc = _NC_CACHE["nc"]
    in_maps = prep_inputs(hidden, adj, a)
    res = run_bass_kernel_spmd(nc, in_maps, list(range(NCORES)), **spmd_kwargs)
    # out[q, v, (pair, c)] -> [b, i, c] ; normalize by denominator col
    outs = []
    for c in range(NCORES):
        o = res.results[c]["out"].astype(np.float32)         # [NOCT, 128, 528]
        o = (o.reshape(NOCT, 2, N, 4, HHW)                   # [q, u, i, pair, c]
             .transpose(0, 3, 1, 2, 4)                       # [q, pair, u, i, c]
             .reshape(BPC, N, HHW))
        outs.append(o[:, :, 0:D] / o[:, :, D:D + 1])
    out = np.concatenate(outs, axis=0)
    return out.reshape(B, N, D).astype(np.float32), res


def kernel(hidden, adj, a):
    out, _ = run_device(hidden, adj, a)
    return out
